# revision 1
# baseline (speedup 1.0000x reference)
"""Axial attention (B=4, H=W=C=64) on 8 trn2 NeuronCores.

Sharding: core k = 2*b + s handles batch b, sequence-half s.
  Phase 1 (height attention): seq = (w,c), features = h.  Core handles
    rows with w in [32s, 32s+32).  All tensors feature-major [64, 4096];
    the host feeds x with the core's own seq columns first.
  Exchange: each core writes its half of x_new transposed to [w, (h,c)]
    layout (scatter DMA, 256B runs) and a paired AllGather assembles the
    full [64 w, 4096 (h,c)] tensor on both cores of the pair.
  Phase 2 (width attention): seq = (h,c), features = w.  The per-core
    column rotation (own rows first) is a data-driven linear combination
    of the two 2048-col panels with host-fed 0/1 per-partition scalars,
    so all 8 cores execute the identical program.

PE packing: the S = Q Q^T matmuls contract over only 64 partitions, so
two j-chunks run concurrently in row groups 0-63 / 64-127 (q duplicated
into both partition halves).  The A@V matmuls have M=64, so two output
windows run concurrently in col groups 0-63 / 64-127 of a shared
[128, 1024] PSUM accumulator.

Math notes:
  q = k, so S is symmetric and S^T tiles (contraction index on
  partitions) feed the A@V matmul directly.  Bias is folded in via an
  augmented ones-row (K=65).  The residual (+x) is an identity matmul
  into the same PSUM accumulator; the per-attention output scale
  (h_weight/w_weight) is folded into the V projection weights on the
  host.
"""

import sys

for _p in ("/opt/trn_rl_repo",):
    if _p not in sys.path:
        sys.path.insert(0, _p)

import numpy as np
import ml_dtypes

import concourse.bass as bass
import concourse.mybir as mybir
import concourse.tile as tile
from concourse import bacc
from concourse import bass_utils
from concourse.bass import ts

F32 = mybir.dt.float32
BF16 = mybir.dt.bfloat16
BF16_NP = ml_dtypes.bfloat16

# If tracing is requested (e.g. BASS_TRACE in the environment) but this
# container's antenv lacks axon_hooks, run_bass_kernel_spmd would crash on
# import.  Provide a null-hook stub so it degrades to an untraced run.
try:
    import antenv.axon_hooks  # noqa: F401
except ImportError:
    import types as _types

    _ah = _types.ModuleType("antenv.axon_hooks")
    _state = {"hook": None}
    _ah.set_axon_ntff_profile_hook = lambda h: _state.__setitem__("hook", h)
    _ah.get_axon_ntff_profile_hook = lambda: _state["hook"]
    sys.modules["antenv.axon_hooks"] = _ah
    try:
        import antenv

        antenv.axon_hooks = _ah
    except ImportError:
        pass

SEQ = 4096   # sequence length per attention (64*64)
HALF = 2048  # rows owned per core
NJ = 32      # 128-row contraction chunks over full seq
REPLICA_GROUPS = [[0, 1], [2, 3], [4, 5], [6, 7]]

_CACHE = {}


def _attention_phase(nc, pools, xaug, q_w, v_w, ident, psum_o, epilogue=None):
    """One axial attention for this core's 2048 rows.

    xaug:  [65, 4096] bf16 SBUF, rows 0-63 = x^T (features x seq, own seq
           cols first), row 64 = ones.
    q_w:   [65, 64] bf16 SBUF = [W_q^T ; b_q]
    v_w:   [65, 64] bf16 SBUF = [W_v^T ; b_v] * out_scale
    psum_o: [128, 1024] f32 PSUM accumulator; window w of the core's four
            512-col output windows lives at
            psum_o[64*(w&1):64*(w&1)+64, (w>>1)*512 : +512].
            On return holds x^T + out_scale * (A @ V)^T.
    """
    ps_pool, p_pool, sb_pool = pools
    Sig = mybir.ActivationFunctionType.Sigmoid

    # residual: psum_o = I^T @ x  (opens the accumulation groups)
    for w in range(4):
        k, h2 = w & 1, w >> 1
        nc.tensor.matmul(
            psum_o[64 * k:64 * k + 64, ts(h2, 512)],
            ident[:], xaug[0:64, ts(w, 512)],
            start=True, stop=False, tile_position=(0, 64 * k),
        )

    # q^T duplicated into both partition halves: [128, 4096] bf16
    qT = sb_pool.tile([128, SEQ], BF16, tag="qT", name="qT")
    for w4 in range(4):
        ps_q = ps_pool.tile([128, 1024], F32, tag="ps", name="ps_q")
        for u in range(2):
            w8 = 2 * w4 + u
            nc.tensor.matmul(ps_q[0:64, ts(u, 512)], q_w[:],
                             xaug[:, ts(w8, 512)], start=True, stop=True)
            nc.tensor.matmul(ps_q[64:128, ts(u, 512)], q_w[:],
                             xaug[:, ts(w8, 512)], start=True, stop=True,
                             tile_position=(0, 64))
        nc.vector.tensor_copy(qT[:, ts(w4, 1024)], ps_q[:])

    # v seq-major: chunk j -> v_sb[:, 64j:64j+64] = V[128j:128j+128, :].
    # Groups are emitted lazily inside the first sweep so the first
    # S-matmul/sigmoid rounds are not queued behind the whole projection.
    v_sb = sb_pool.tile([128, NJ * 64], BF16, tag="v_sb", name="v_sb")

    def emit_v_group(g):
        ps_v = ps_pool.tile([128, 512], F32, tag="ps", name="ps_v")
        for u in range(8):
            j = 8 * g + u
            nc.tensor.matmul(ps_v[:, ts(u, 64)], xaug[:, ts(j, 128)], v_w[:],
                             start=True, stop=True)
        nc.vector.tensor_copy(v_sb[:, ts(g, 512)], ps_v[:])

    # main loop: S^T tiles -> sigmoid -> A@V, output bank h2 completed
    # per outer sweep so its epilogue (store + AllGather chunk) overlaps
    # the other sweep's compute.
    # Each PSUM tile gets one row-group-0 (j0) and one row-group-64 (j1)
    # matmul so the pair shares one slot dependency and the scheduler
    # keeps them adjacent -> the two MMs run concurrently in the array
    # (and a full-array pair keeps the PE clock warm; solo K=64 MMs run
    # permanently cold at half rate).
    for g in range(4):
        emit_v_group(g)

    for h2 in range(2):
        for jp in range(NJ // 2):
            j0, j1 = 2 * jp, 2 * jp + 1
            last = jp == NJ // 2 - 1
            pair = []
            for k in range(2):
                win = bass.ds(h2 * 1024 + k * 512, 512)
                ps_k = ps_pool.tile([128, 1024], F32, tag="ps", name="ps_k")
                nc.tensor.matmul(ps_k[:, 0:512], qT[0:64, ts(j0, 128)],
                                 qT[0:64, win], start=True, stop=True)
                nc.tensor.matmul(ps_k[:, 512:1024], qT[64:128, ts(j1, 128)],
                                 qT[64:128, win], start=True, stop=True)
                p_k = p_pool.tile([128, 1024], BF16, tag="p", name="p_k")
                nc.scalar.activation(p_k[:], ps_k[:], Sig, scale=0.125)
                pair.append(p_k)
            # col-packed A@V: window w=2*h2+k -> psum_o[64k:64k+64, h2*512:]
            for ji, (j, off) in enumerate(((j0, 0), (j1, 512))):
                for k in range(2):
                    nc.tensor.matmul(
                        psum_o[64 * k:64 * k + 64, ts(h2, 512)],
                        v_sb[:, ts(j, 64)],
                        pair[k][:, bass.ds(off, 512)],
                        start=False, stop=(last and ji == 1),
                        tile_position=(0, 64 * k),
                    )
        if epilogue is not None:
            epilogue(h2)


def _build():
    nc = bacc.Bacc("TRN2", target_bir_lowering=False, debug=False,
                   num_devices=8)

    x16_d = nc.dram_tensor("x16aug", [65, SEQ], BF16, kind="ExternalInput")
    hq_d = nc.dram_tensor("hq_aug", [65, 64], BF16, kind="ExternalInput")
    hv_d = nc.dram_tensor("hv_aug", [65, 64], BF16, kind="ExternalInput")
    wq_d = nc.dram_tensor("wq_aug", [65, 64], BF16, kind="ExternalInput")
    wv_d = nc.dram_tensor("wv_aug", [65, 64], BF16, kind="ExternalInput")
    id_d = nc.dram_tensor("ident", [64, 64], BF16, kind="ExternalInput")
    sel_d = nc.dram_tensor("sel", [64, 2], F32, kind="ExternalInput")
    out_d = nc.dram_tensor("out", [32, 64, 64], F32, kind="ExternalOutput")

    with tile.TileContext(nc) as tc:
        with (
            tc.tile_pool(name="consts", bufs=1) as cpool,
            tc.tile_pool(name="sb", bufs=1) as sb_pool,
            tc.tile_pool(name="ptiles", bufs=4) as p_pool,
            tc.tile_pool(name="ps", bufs=3, space="PSUM") as ps_pool,
            tc.tile_pool(name="pso", bufs=1, space="PSUM") as pso_pool,
            tc.tile_pool(name="dram", bufs=1, space="DRAM") as dram_pool,
        ):
            # constants
            hq = cpool.tile([65, 64], BF16, name="hq")
            hv = cpool.tile([65, 64], BF16, name="hv")
            wq = cpool.tile([65, 64], BF16, name="wq")
            wv = cpool.tile([65, 64], BF16, name="wv")
            ident = cpool.tile([64, 64], BF16, name="ident")
            sel = cpool.tile([64, 2], F32, name="sel")
            for t, d in ((hq, hq_d), (hv, hv_d), (wq, wq_d), (wv, wv_d),
                         (ident, id_d), (sel, sel_d)):
                nc.sync.dma_start(t[:], d[:])

            # warm the sigmoid table set early (hides the ~2.7us table load)
            warm = cpool.tile([128, 16], BF16, name="warm")
            nc.vector.memset(warm[:], 0.0)
            nc.scalar.activation(
                warm[:], warm[:], mybir.ActivationFunctionType.Sigmoid
            )

            pools = (ps_pool, p_pool, sb_pool)

            # ---------------- phase 1: height attention ----------------
            # spread the input load across four engines' DMA queues
            x16 = sb_pool.tile([65, SEQ], BF16, tag="x16", name="x16")
            dma_engs = (nc.sync, nc.scalar, nc.gpsimd)
            for q8 in range(8):
                dma_engs[q8 % 3].dma_start(x16[:, ts(q8, 512)],
                                           x16_d[:, ts(q8, 512)])

            pso1 = pso_pool.tile([128, 1024], F32, tag="pso", name="pso1")

            # exchange buffers: cc_in [wl, h, c] own transposed half; two
            # AllGather chunks (wl halves) so chunk 0 overlaps the h2=1
            # compute sweep.  ccA/ccB = [2 ranks, 16 wl, (h c)].
            xnew1 = sb_pool.tile([128, 1024], BF16, tag="xnew1", name="xnew1")
            cc_in = dram_pool.tile([32, 64, 64], BF16, name="cc_in")
            cc_a = dram_pool.tile([2, 16, SEQ], BF16, name="cc_a")
            cc_b = dram_pool.tile([2, 16, SEQ], BF16, name="cc_b")
            cc_in_r = cc_in[:].rearrange("wl h c -> h wl c")

            def epi1(h2):
                nc.vector.tensor_copy(xnew1[:, ts(h2, 512)],
                                      pso1[:, ts(h2, 512)])
                for k in range(2):
                    w = 2 * h2 + k
                    src = xnew1[64 * k:64 * k + 64, ts(h2, 512)]
                    src_v = src.rearrange("h (wl c) -> h wl c", c=64)
                    nc.sync.dma_start(cc_in_r[:, ts(w, 8), :], src_v)
                nc.gpsimd.collective_compute(
                    "AllGather",
                    mybir.AluOpType.bypass,
                    replica_groups=REPLICA_GROUPS,
                    ins=[cc_in[bass.ds(16 * h2, 16), :, :].opt()],
                    outs=[(cc_a if h2 == 0 else cc_b)[:].opt()],
                )

            _attention_phase(nc, pools, x16, hq, hv, ident, pso1,
                             epilogue=epi1)

            # ---------------- phase 2: width attention -----------------
            # x2stage rows w: 0-15 <- ccA[0], 16-31 <- ccB[0],
            #                 32-47 <- ccA[1], 48-63 <- ccB[1]
            x2stage = sb_pool.tile([64, SEQ], BF16, tag="x2stage",
                                   name="x2stage")
            for blk, src_t in (((0, 0), cc_a), ((1, 0), cc_b),
                               ((2, 1), cc_a), ((3, 1), cc_b)):
                q4, rank = blk
                dma_engs[q4 % 2].dma_start(x2stage[bass.ds(16 * q4, 16), :],
                                           src_t[rank, :, :])

            # panel select: own (h,c) rows first, via host-fed 0/1 scalars;
            # chunked so phase-2 projections can start early
            x2aug = sb_pool.tile([65, SEQ], BF16, tag="x2aug", name="x2aug")
            nc.vector.memset(x2aug[64:65, :], 1.0)
            sa = sel[:, 0:1]
            sb = sel[:, 1:2]
            for half in range(2):
                c0 = sa if half == 0 else sb
                c1 = sb if half == 0 else sa
                for q2 in range(2):
                    t0 = sb_pool.tile([64, 1024], BF16, tag="selt0", name="t0")
                    t1 = sb_pool.tile([64, 1024], BF16, tag="selt1", name="t1")
                    nc.vector.tensor_scalar_mul(
                        t0[:], x2stage[:, bass.ds(q2 * 1024, 1024)], c0)
                    nc.vector.tensor_scalar_mul(
                        t1[:], x2stage[:, bass.ds(HALF + q2 * 1024, 1024)], c1)
                    nc.vector.tensor_add(
                        x2aug[0:64, bass.ds(half * HALF + q2 * 1024, 1024)],
                        t0[:], t1[:]
                    )

            pso2 = pso_pool.tile([128, 1024], F32, tag="pso", name="pso2")
            xnew2 = sb_pool.tile([128, 1024], F32, tag="xnew2", name="xnew2")
            out_r = out_d[:].rearrange("hl w c -> w hl c")

            def epi2(h2):
                # final store: window w holds (hl,c) cols [512w : 512w+512)
                nc.vector.tensor_copy(xnew2[:, ts(h2, 512)],
                                      pso2[:, ts(h2, 512)])
                for k in range(2):
                    w = 2 * h2 + k
                    src = xnew2[64 * k:64 * k + 64, ts(h2, 512)]
                    src_v = src.rearrange("w (hl c) -> w hl c", c=64)
                    nc.sync.dma_start(out_r[:, ts(w, 8), :], src_v)

            _attention_phase(nc, pools, x2aug, wq, wv, ident, pso2,
                             epilogue=epi2)

    nc.compile()
    return nc


def _get_nc():
    if "nc" not in _CACHE:
        _CACHE["nc"] = _build()
    return _CACHE["nc"]


def kernel(x, hq_w, hq_b, hv_w, hv_b, wq_w, wq_b, wv_w, wv_b,
           h_weight, w_weight, **kwargs):
    x = np.asarray(x, np.float32)
    fp = lambda a: np.asarray(a, np.float32)

    hq_aug = np.concatenate([fp(hq_w).T, fp(hq_b)[None, :]], 0).astype(BF16_NP)
    wq_aug = np.concatenate([fp(wq_w).T, fp(wq_b)[None, :]], 0).astype(BF16_NP)
    hv_aug = (np.concatenate([fp(hv_w).T, fp(hv_b)[None, :]], 0)
              * fp(h_weight)[0]).astype(BF16_NP)
    wv_aug = (np.concatenate([fp(wv_w).T, fp(wv_b)[None, :]], 0)
              * fp(w_weight)[0]).astype(BF16_NP)
    ident = np.eye(64, dtype=np.float32).astype(BF16_NP)
    ones_row = np.ones((1, SEQ), np.float32)

    in_maps = []
    for b in range(4):
        xb = x[b].reshape(64, SEQ)  # [h, (w,c)]
        for s in range(2):
            xp = xb if s == 0 else np.concatenate(
                [xb[:, HALF:], xb[:, :HALF]], axis=1
            )
            x16aug = np.concatenate([xp, ones_row], 0).astype(BF16_NP)
            selv = np.zeros((64, 2), np.float32)
            selv[:, s] = 1.0
            in_maps.append({
                "x16aug": np.ascontiguousarray(x16aug),
                "hq_aug": hq_aug, "hv_aug": hv_aug,
                "wq_aug": wq_aug, "wv_aug": wv_aug,
                "ident": ident, "sel": selv,
            })

    nc = _get_nc()
    res = bass_utils.run_bass_kernel_spmd(
        nc, in_maps, core_ids=list(range(8)), **kwargs
    )
    _CACHE["last_result"] = res

    out = np.empty((4, 64, 64, 64), np.float32)
    for b in range(4):
        for s in range(2):
            out[b, 32 * s:32 * s + 32] = res.results[2 * b + s]["out"]
    return out


def last_exec_time_ns():
    res = _CACHE.get("last_result")
    return None if res is None else res.exec_time_ns

